# revision 1
# baseline (speedup 1.0000x reference)
"""ConflateLoss Trainium2 kernel.

loss = -sum_b log_softmax(10 * cos_sim(anchor_b, match[cand_idx_b]))[0] / ln(10)
with B=16384, D=128, 50 candidates per anchor (slot 0 = positive b, 1..49 = neg_idx).

Strategy (8 NeuronCores, data-parallel over B):
  Phase 0 (each core): normalize match_embedding rows to unit L2 norm, cast to
    bf16, stage to a private DRAM buffer `mhat`. Normalize this core's 2048
    anchors by 10/||a|| (gamma folded) into SBUF bf16.
  Phase 1: per 128-anchor block, one dma_gather pulls the 6400 candidate rows
    (bf16, 256B each) from mhat into SBUF laid out [anchor_part, slot, d].
    DVE: prod = cand * anchor (broadcast over slots), two-stage add-reduce over
    d -> logits f32 [128, 50]. ACT: Exp with accum_out -> softmax denominator
    in one instruction, Ln, then (ln(denom) - logit0) per anchor.
  Output: [128, 16] per-core partial losses; host sums and divides by ln(10).
"""

import math

import numpy as np

import concourse.bacc as bacc
import concourse.bass as bass
import concourse.tile as tile
from concourse import mybir
from concourse.bass_utils import run_bass_kernel_spmd

B = 16384
D = 128
N_NEG = 49
N_CAND = 50
N_CORES = 8
B_SHARD = B // N_CORES  # 2048 anchors per core
N_BLK = B_SHARD // 128  # 16 blocks of 128 anchors
N_IDX = 128 * N_CAND  # 6400 gathered rows per block
IDX_COLS = N_IDX // 16  # 400 (dma_gather index wrap: token i at [i%16, i//16])
M_GROUPS = 16  # match rows processed in 16 groups of 1024
G_ROWS = B // M_GROUPS // 128  # 8 rows per partition per group

F32 = mybir.dt.float32
BF16 = mybir.dt.bfloat16
AF = mybir.ActivationFunctionType


def _bcast_mid(ap: bass.AP, count: int) -> bass.AP:
    """[128, D] AP -> [128, count, D] with a stride-0 middle dim."""
    assert len(ap.ap) == 2
    return bass.AP(
        tensor=ap.tensor,
        offset=ap.offset,
        ap=[list(ap.ap[0]), [0, count], list(ap.ap[1])],
    )


def build_bass() -> bacc.Bacc:
    nc = bacc.Bacc("TRN2", debug=False, num_devices=N_CORES)

    match_in = nc.dram_tensor("match", [B, D], F32, kind="ExternalInput")
    anchors_in = nc.dram_tensor("anchors", [B_SHARD, D], F32, kind="ExternalInput")
    idx_in = nc.dram_tensor("idx", [128, N_BLK, IDX_COLS], mybir.dt.int16,
                            kind="ExternalInput")
    out = nc.dram_tensor("out", [128, N_BLK], F32, kind="ExternalOutput")

    with tile.TileContext(nc) as tc:
        with (
            tc.tile_pool(name="dram", bufs=1, space="DRAM") as dram_pool,
            tc.tile_pool(name="mload", bufs=2) as mload,
            tc.tile_pool(name="msq", bufs=2) as msq,
            tc.tile_pool(name="mnorm", bufs=3) as mnorm,
            tc.tile_pool(name="mhatsb", bufs=2) as mhatsb,
            tc.tile_pool(name="single", bufs=1) as single,
            tc.tile_pool(name="cand", bufs=3) as candp,
            tc.tile_pool(name="prod", bufs=2) as prodp,
            tc.tile_pool(name="small", bufs=4) as small,
        ):
            mhat = dram_pool.tile([B, D], BF16)
            # row r of mhat/match handled by partition (r % 1024) // 8 of
            # group r // 1024 -> per-partition-contiguous DMA both ways.
            match_r = match_in.ap().rearrange(
                "(g p t) d -> g p t d", p=128, t=G_ROWS)
            mhat_r = mhat[:].rearrange("(g p t) d -> g p t d", p=128, t=G_ROWS)

            # ---- Phase 0a: normalize match rows -> mhat (bf16, unit norm)
            for g in range(M_GROUPS):
                mf = mload.tile([128, G_ROWS, D], F32)
                nc.sync.dma_start(out=mf, in_=match_r[g])
                sq = msq.tile([128, G_ROWS, D], F32)
                nc.scalar.activation(out=sq, in_=mf, func=AF.Square)
                nsq = mnorm.tile([128, G_ROWS], F32)
                nc.vector.tensor_reduce(
                    out=nsq, in_=sq, axis=mybir.AxisListType.X,
                    op=mybir.AluOpType.add)
                rin = mnorm.tile([128, G_ROWS], F32)
                nc.vector.reciprocal(out=rin, in_=nsq)
                inv = mnorm.tile([128, G_ROWS], F32)
                nc.scalar.activation(out=inv, in_=rin, func=AF.Sqrt)
                mh = mhatsb.tile([128, G_ROWS, D], BF16)
                for t in range(G_ROWS):
                    nc.scalar.mul(out=mh[:, t, :], in_=mf[:, t, :],
                                  mul=inv[:, t:t + 1])
                nc.sync.dma_start(out=mhat_r[g], in_=mh)

            # ---- Phase 0b: anchors -> bf16, scaled by 10/||a||
            anch_r = anchors_in.ap().rearrange("(k p) d -> p k d", p=128)
            af = single.tile([128, N_BLK, D], F32)
            nc.sync.dma_start(out=af, in_=anch_r)
            asq = msq.tile([128, N_BLK, D], F32, tag="msq")
            nc.scalar.activation(out=asq, in_=af, func=AF.Square)
            nsqa = single.tile([128, N_BLK], F32)
            nc.vector.tensor_reduce(
                out=nsqa, in_=asq, axis=mybir.AxisListType.X,
                op=mybir.AluOpType.add)
            ra = single.tile([128, N_BLK], F32)
            nc.vector.reciprocal(out=ra, in_=nsqa)
            sca = single.tile([128, N_BLK], F32)
            # sqrt(100/nsq) = 10/||a||  (gamma=10 folded into the anchor)
            nc.scalar.activation(out=sca, in_=ra, func=AF.Sqrt, scale=100.0)
            ab = single.tile([128, N_BLK, D], BF16)
            for k in range(N_BLK):
                nc.scalar.mul(out=ab[:, k, :], in_=af[:, k, :],
                              mul=sca[:, k:k + 1])

            # ---- index table (tokens pre-wrapped host-side)
            idxs = single.tile([128, N_BLK, IDX_COLS], mybir.dt.int16)
            nc.sync.dma_start(out=idxs, in_=idx_in.ap())

            lossacc = single.tile([128, N_BLK], F32)

            # ---- Phase 1: gather + dots + softmax per 128-anchor block
            for k in range(N_BLK):
                cand = candp.tile([128, N_CAND, D], BF16)
                nc.gpsimd.dma_gather(
                    cand[:], mhat[:], idxs[:, k, :], N_IDX, N_IDX, D,
                    single_packet=False)
                prod = prodp.tile([128, N_CAND, D], BF16)
                nc.vector.tensor_tensor(
                    out=prod, in0=cand, in1=_bcast_mid(ab[:, k, :], N_CAND),
                    op=mybir.AluOpType.mult)
                # single-stage reduce: fp32-internal accumulation; 1-port DVE
                # mode (2-port modes would stall gpsimd SWDGE ring writes).
                logits = small.tile([128, N_CAND], F32)
                nc.vector.tensor_reduce(
                    out=logits, in_=prod, axis=mybir.AxisListType.X,
                    op=mybir.AluOpType.add)
                escr = small.tile([128, N_CAND], BF16)
                den = small.tile([128, 1], F32)
                # |logit| <= 10 so exp never overflows; skip max-subtraction.
                nc.scalar.activation(out=escr, in_=logits, func=AF.Exp,
                                     accum_out=den)
                lden = small.tile([128, 1], F32)
                nc.scalar.activation(out=lden, in_=den, func=AF.Ln)
                nc.vector.tensor_tensor(
                    out=lossacc[:, k:k + 1], in0=lden, in1=logits[:, 0:1],
                    op=mybir.AluOpType.subtract)

            nc.sync.dma_start(out=out.ap(), in_=lossacc)

    nc.compile()
    return nc


def make_in_maps(anchor_embedding, match_embedding, neg_idx):
    match = np.ascontiguousarray(np.asarray(match_embedding), dtype=np.float32)
    anchors = np.ascontiguousarray(np.asarray(anchor_embedding), dtype=np.float32)
    nidx = np.asarray(neg_idx).astype(np.int64)

    i = np.arange(N_IDX)
    b_l, n = i % 128, i // 128  # token i = n*128 + b -> dst[b, n, :]
    in_maps = []
    for c in range(N_CORES):
        lo = c * B_SHARD
        cand_idx = np.concatenate(
            [np.arange(lo, lo + B_SHARD, dtype=np.int64)[:, None],
             nidx[lo:lo + B_SHARD]], axis=1).astype(np.int16)  # [2048, 50]
        toks = np.empty((N_BLK, N_IDX), np.int16)
        for k in range(N_BLK):
            toks[k] = cand_idx[k * 128 + b_l, n]
        # dma_gather index wrap: token i read from [i % 16, i // 16],
        # replicated into each 16-partition group (one per gpsimd core).
        sb = toks.reshape(N_BLK, IDX_COLS, 16).transpose(2, 0, 1)  # [16,k,s]
        idx_host = np.tile(sb, (8, 1, 1))
        in_maps.append({
            "match": match,
            "anchors": anchors[lo:lo + B_SHARD],
            "idx": idx_host,
        })
    return in_maps


_NC_CACHE = None


def kernel(anchor_embedding, match_embedding, neg_idx) -> np.ndarray:
    global _NC_CACHE
    if _NC_CACHE is None:
        _NC_CACHE = build_bass()
    nc = _NC_CACHE
    in_maps = make_in_maps(anchor_embedding, match_embedding, neg_idx)
    res = run_bass_kernel_spmd(nc, in_maps, core_ids=list(range(N_CORES)))
    total = sum(float(r["out"].astype(np.float64).sum()) for r in res.results)
    return np.asarray(total / math.log(10.0), dtype=np.float32)



# revision 2
# speedup vs baseline: 9.5916x; 9.5916x over previous
"""ConflateLoss Trainium2 kernel — single-blob I/O, lean phase0.

loss = -sum_b log_softmax(10 * cos_sim(anchor_b, match[cand_idx_b]))[0] / ln(10)
B=16384, D=128, 50 candidates per anchor (slot 0 = positive b, 1..49 = neg_idx).

Per-call cost in this axon-tunneled environment is dominated by fixed
dispatch overhead plus per-input-buffer and per-byte relay costs, so this
kernel ships ONE packed int16 input per core (match bf16 4MB + anchors bf16
0.5MB + compact gather tokens 0.2MB) instead of three f32/int64 tensors,
and keeps the device program short:

  Phase 0: normalize match rows -> mhat (bf16, unit norm) in DRAM, 4 chunks;
    big-tile DVE broadcast-multiply instead of per-row scalar muls.
    Anchors scaled by 10/||a|| (gamma folded), bf16.
  Phase 1: 8 groups of 256 anchors; per group one dma_gather pulls 12800
    candidate rows (bf16, 256B each) from mhat; DVE mult+reduce -> logits;
    ACT Exp, DVE reduce -> softmax denom; Ln; subtract logit0.
  Output: [128, 16] per-core partials; host sums / ln(10).
"""

import math

import numpy as np

import concourse.bacc as bacc
import concourse.bass as bass
import concourse.tile as tile
from concourse import mybir
from concourse.bass_utils import run_bass_kernel_spmd

B = 16384
D = 128
N_NEG = 49
N_CAND = 50
N_CORES = 8
B_SHARD = B // N_CORES      # 2048 anchors per core
N_BLK = B_SHARD // 128      # 16 blocks of 128 anchors
N_GRP = 8                   # gather groups: 2 blocks (256 anchors) each
G_IDX = 256 * N_CAND        # 12800 gathered rows per group
G_COLS = G_IDX // 16        # 800 (dma_gather token wrap)
M_CHUNK = 4                 # phase-0 match chunks
T_CHUNK = B // 128 // M_CHUNK  # 32 rows per partition per chunk

ROWS_MATCH = B              # blob rows 0..16383: match bf16
ROWS_ANCH = B_SHARD         # blob rows 16384..18431: anchors bf16 (p*16+k)
ROWS_IDX = N_GRP * G_COLS * 16 // 128  # 800
ROW_A0 = ROWS_MATCH
ROW_I0 = ROWS_MATCH + ROWS_ANCH
ROWS_TOTAL = ROW_I0 + ROWS_IDX  # 19232

F32 = mybir.dt.float32
BF16 = mybir.dt.bfloat16
I16 = mybir.dt.int16
AF = mybir.ActivationFunctionType


def _bcast_last(ap: bass.AP, count: int) -> bass.AP:
    """[128, t] AP -> [128, t, count] with a stride-0 last dim."""
    return bass.AP(tensor=ap.tensor, offset=ap.offset,
                   ap=[list(d) for d in ap.ap] + [[0, count]])


def _drop_last1(ap: bass.AP) -> bass.AP:
    """[..., 1] AP -> drop the trailing unit dim."""
    assert ap.ap[-1][1] == 1
    return bass.AP(tensor=ap.tensor, offset=ap.offset,
                   ap=[list(d) for d in ap.ap[:-1]])


def build_bass() -> bacc.Bacc:
    nc = bacc.Bacc("TRN2", debug=False, num_devices=N_CORES,
                   enable_partition_id=False)

    blob = nc.dram_tensor("blob", [ROWS_TOTAL, 128], I16,
                          kind="ExternalInput")
    out = nc.dram_tensor("out", [128, N_BLK], F32, kind="ExternalOutput")

    match_sec = blob[0:ROWS_MATCH, :].bitcast(BF16)
    anch_sec = blob[ROW_A0:ROW_I0, :].bitcast(BF16)

    with tile.TileContext(nc) as tc:
        with (
            tc.tile_pool(name="dram", bufs=1, space="DRAM") as dram_pool,
            tc.tile_pool(name="mload", bufs=2) as mload,
            tc.tile_pool(name="msq", bufs=2) as msq,
            tc.tile_pool(name="mnorm", bufs=2) as mnorm,
            tc.tile_pool(name="mhatsb", bufs=2) as mhatsb,
            tc.tile_pool(name="single", bufs=1) as single,
            tc.tile_pool(name="cand", bufs=2) as candp,
            tc.tile_pool(name="prod", bufs=1) as prodp,
            tc.tile_pool(name="small", bufs=4) as small,
        ):
            mhat = dram_pool.tile([B, D], BF16)
            # row r = p*128 + c*32 + t  ->  partition p, chunk c, slot t:
            # per-partition contiguous 8KB both ways.
            match_r = match_sec.rearrange("(p c t) d -> c p t d",
                                          p=128, c=M_CHUNK)
            mhat_r = mhat[:].rearrange("(p c t) d -> c p t d",
                                       p=128, c=M_CHUNK)

            # ---- Phase 0a: normalize match rows -> mhat (bf16, unit norm)
            for c in range(M_CHUNK):
                mf = mload.tile([128, T_CHUNK, D], BF16)
                nc.sync.dma_start(out=mf, in_=match_r[c])
                sq = msq.tile([128, T_CHUNK, D], BF16)
                nc.scalar.activation(out=sq, in_=mf, func=AF.Square)
                nsq = mnorm.tile([128, T_CHUNK], F32)
                nc.vector.tensor_reduce(
                    out=nsq, in_=sq, axis=mybir.AxisListType.X,
                    op=mybir.AluOpType.add)
                rin = mnorm.tile([128, T_CHUNK], F32)
                nc.vector.reciprocal(out=rin, in_=nsq)
                inv = mnorm.tile([128, T_CHUNK], BF16)
                nc.scalar.activation(out=inv, in_=rin, func=AF.Sqrt)
                mh = mhatsb.tile([128, T_CHUNK, D], BF16)
                nc.vector.tensor_tensor(
                    out=mh, in0=mf, in1=_bcast_last(inv[:], D),
                    op=mybir.AluOpType.mult)
                nc.sync.dma_start(out=mhat_r[c], in_=mh)

            # ---- Phase 0b: anchors -> bf16 scaled by 10/||a||
            af = single.tile([128, N_BLK, D], BF16)
            nc.sync.dma_start(
                out=af, in_=anch_sec.rearrange("(p k) d -> p k d", p=128))
            asq = msq.tile([128, N_BLK, D], BF16, tag="msq")
            nc.scalar.activation(out=asq, in_=af, func=AF.Square)
            nsqa = single.tile([128, N_BLK], F32)
            nc.vector.tensor_reduce(
                out=nsqa, in_=asq, axis=mybir.AxisListType.X,
                op=mybir.AluOpType.add)
            ra = single.tile([128, N_BLK], F32)
            nc.vector.reciprocal(out=ra, in_=nsqa)
            sca = single.tile([128, N_BLK], BF16)
            # sqrt(100/nsq) = 10/||a||  (gamma=10 folded into the anchor)
            nc.scalar.activation(out=sca, in_=ra, func=AF.Sqrt, scale=100.0)
            ab = single.tile([128, N_BLK, D], BF16)
            nc.vector.tensor_tensor(
                out=ab, in0=af, in1=_bcast_last(sca[:], D),
                op=mybir.AluOpType.mult)

            # ---- token table: compact [16, N_GRP*G_COLS] in DRAM,
            # replicated on device to all 8 gpsimd-core partition groups
            # (DMA pairs src/dst elements in iteration order).
            idxs = single.tile([128, N_GRP, G_COLS], I16)
            idx_src = bass.AP(
                tensor=blob, offset=ROW_I0 * 128,
                ap=[[0, 8], [N_GRP * G_COLS, 16], [1, N_GRP * G_COLS]])
            nc.sync.dma_start(out=idxs, in_=idx_src)

            lossacc = single.tile([128, N_BLK], F32)

            # ---- Phase 1: gather + dots + softmax, 2 blocks per group
            for g in range(N_GRP):
                cand = candp.tile([128, 2 * N_CAND, D], BF16)
                nc.gpsimd.dma_gather(
                    cand[:], mhat[:], idxs[:, g, :], G_IDX, G_IDX, D,
                    single_packet=False)
                prod = prodp.tile([128, 2 * N_CAND, D], BF16)
                # anchors for blocks 2g, 2g+1 broadcast over slots:
                # ab[:, 2g:2g+2, :] viewed [128, 2, 1, D] stride-0 slot dim
                abg = ab[:, 2 * g:2 * g + 2, :]
                ab_b = bass.AP(
                    tensor=abg.tensor, offset=abg.offset,
                    ap=[list(abg.ap[0]), list(abg.ap[1]),
                        [0, N_CAND], list(abg.ap[2])])
                prod4 = prod[:].rearrange("p (h s) d -> p h s d", h=2)
                nc.vector.tensor_tensor(
                    out=prod4, in0=cand[:].rearrange(
                        "p (h s) d -> p h s d", h=2),
                    in1=ab_b, op=mybir.AluOpType.mult)
                logits = small.tile([128, 2, N_CAND], F32)
                nc.vector.tensor_reduce(
                    out=logits, in_=prod4, axis=mybir.AxisListType.X,
                    op=mybir.AluOpType.add)
                esc = small.tile([128, 2, N_CAND], BF16)
                # |logit| <= 10 so exp never overflows; skip max-subtraction.
                nc.scalar.activation(out=esc, in_=logits, func=AF.Exp)
                den = small.tile([128, 2], F32)
                nc.vector.tensor_reduce(
                    out=den, in_=esc, axis=mybir.AxisListType.X,
                    op=mybir.AluOpType.add)
                lden = small.tile([128, 2], F32)
                nc.scalar.activation(out=lden, in_=den, func=AF.Ln)
                nc.vector.tensor_tensor(
                    out=lossacc[:, 2 * g:2 * g + 2], in0=lden,
                    in1=_drop_last1(logits[:, :, 0:1]),
                    op=mybir.AluOpType.subtract)

            nc.sync.dma_start(out=out.ap(), in_=lossacc)

    nc.compile()
    return nc


def make_in_maps(anchor_embedding, match_embedding, neg_idx):
    bf16 = mybir.dt.np(BF16)
    match = np.asarray(match_embedding, dtype=np.float32)
    anchors = np.asarray(anchor_embedding, dtype=np.float32)
    nidx = np.asarray(neg_idx).astype(np.int64)

    match_i16 = np.ascontiguousarray(match.astype(bf16)).view(np.int16)

    # token i (of 12800, per group) -> dst[b, n]: b = i % 128, n = i // 128;
    # n = h*50 + s -> anchor (g*256 + h*128 + b), slot s.
    i = np.arange(G_IDX)
    b_l, n = i % 128, i // 128
    h, s = n // N_CAND, n % N_CAND

    in_maps = []
    for c in range(N_CORES):
        lo = c * B_SHARD
        cand_idx = np.concatenate(
            [np.arange(lo, lo + B_SHARD, dtype=np.int64)[:, None],
             nidx[lo:lo + B_SHARD]], axis=1).astype(np.int16)  # [2048, 50]
        toks = np.empty((N_GRP, G_IDX), np.int16)
        for g in range(N_GRP):
            toks[g] = cand_idx[g * 256 + h * 128 + b_l, s]
        # wrap: token j at [j % 16, j // 16]  -> [16, N_GRP, G_COLS]
        sb = toks.reshape(N_GRP, G_COLS, 16).transpose(2, 0, 1)

        anch = anchors[lo:lo + B_SHARD].astype(bf16)
        # blob row ROW_A0 + p*16 + k holds anchor (k*128 + p)
        anch_pk = anch.reshape(N_BLK, 128, D).transpose(1, 0, 2)

        blob = np.concatenate([
            match_i16,
            np.ascontiguousarray(anch_pk).reshape(B_SHARD, D).view(np.int16),
            np.ascontiguousarray(sb).reshape(ROWS_IDX, 128),
        ], axis=0)
        in_maps.append({"blob": blob})
    return in_maps


_NC_CACHE = None


def kernel(anchor_embedding, match_embedding, neg_idx) -> np.ndarray:
    global _NC_CACHE
    if _NC_CACHE is None:
        _NC_CACHE = build_bass()
    nc = _NC_CACHE
    in_maps = make_in_maps(anchor_embedding, match_embedding, neg_idx)
    res = run_bass_kernel_spmd(nc, in_maps, core_ids=list(range(N_CORES)))
    total = sum(float(r["out"].astype(np.float64).sum()) for r in res.results)
    return np.asarray(total / math.log(10.0), dtype=np.float32)
